# revision 41
# baseline (speedup 1.0000x reference)
import os
import sys

import numpy as np

sys.path.insert(0, "/opt/trn_rl_repo")

import ml_dtypes
import concourse.bass as bass
from concourse import bacc
import concourse.mybir as mybir
import concourse.tile as tile
from concourse.bass_utils import run_bass_kernel_spmd

# Problem constants (hardcoded per contract)
B, L, N, H, HU = 512, 16, 10000, 128, 128
NCORES = 8
BL = B // NCORES            # 64 local batch rows per core
T2 = 2 * L                  # 32 node/coord time steps
COLS = T2 * BL              # 2048 columns, t-major: col = t*BL + b
LCOLS = L * BL              # 1024 tau columns
NBLK = 4                    # node GEMM column blocks of 512
NKT4 = 20                   # groups of 4 k-tiles per DMA (80 k-tiles, padded)
NKTP = 4 * NKT4             # 80
NPAD = NKTP * 128           # 10240

F32 = mybir.dt.float32
BF16 = mybir.dt.bfloat16
NPBF = ml_dtypes.bfloat16

SIG = mybir.ActivationFunctionType.Sigmoid
TANH = mybir.ActivationFunctionType.Tanh
IDENT = mybir.ActivationFunctionType.Identity

# bf16 packed constants (matmul operands), column offsets
C_WC = 0
C_WTAU = 128
C_WX2 = 256
C_WRES = 384
C_WE2 = 512
C_WX1 = 640
C_WE1 = 641
C_W2 = 642            # [128, 7]
C_XIN = 649           # [128, 64]
C_T0 = 713
C_END = 777
C_TAU = 841           # [1, 1024]
C_COORDS = 1865       # [2, 2048]
C_IND = 3913          # [4, 512] gate indicator (256-periodic) for bias matmul
C_BK = 4425           # [4, 7*128] per-LSTM gate-bias rows (lhsT per k)
CPW = C_BK + 7 * 128

# fp32 packed biases, column offsets
Z_BTAU = 0
Z_BX2 = 1
Z_BRES = 2
Z_BE2 = 3
Z_B1 = 4              # [128, 7]
Z_B2 = 11             # [1, 7]
CBW = 18

# gate order in PSUM banks: i, f, o | g  (sigmoid on first 192 cols, tanh on last)
GPERM = [0, 1, 3, 2]  # torch (i,f,g,o) -> packed (i,f,o,g)

# symbolic sequence descriptors: (source, index)
def _mk_seqs():
    pre = [("x", 0), ("t0", 0)]
    suf = [("end", 0)]
    seqs = {}
    seqs[0] = pre + [(s, t) for l in range(L)
                     for s, t in (("tau", l), ("node", 2 * l), ("coord", 2 * l),
                                  ("node", 2 * l + 1), ("coord", 2 * l + 1))] + suf
    seqs[1] = pre + [("tau", l) for l in range(L)] + suf
    seqs[2] = [("node", t) for t in range(T2)]
    seqs[3] = [("coord", t) for t in range(T2)]
    seqs[4] = pre + [(s, t) for l in range(L)
                     for s, t in (("tau", l), ("node", 2 * l),
                                  ("node", 2 * l + 1))] + suf
    seqs[5] = [(s, t) for l in range(L)
               for s, t in (("node", 2 * l), ("coord", 2 * l),
                            ("node", 2 * l + 1), ("coord", 2 * l + 1))]
    seqs[6] = pre + [(s, t) for l in range(L)
                     for s, t in (("tau", l), ("coord", 2 * l),
                                  ("coord", 2 * l + 1))] + suf
    return seqs

SEQS = _mk_seqs()
# LSTMs that need a materialized (gathered) sequence buffer; 2/3 read
# nodeh/coordh directly (pairs never straddle a block: boundaries are even).
SEQ_MAT = [0, 1, 4, 5, 6]

# node GEMM column blocks as (t_start, t_end): small blocks first so the
# recurrences get their first node columns early. Every DMA moves a uniform
# [128, 2048] tile (4KB rows, full stream rate) holding 2048/w k-tile slices.
BLOCKS = [(0, 2), (2, 4), (4, 8), (8, 16), (16, 24), (24, 32)]
NBLKV = len(BLOCKS)
BLOCK_OF = {}
for _j, (_a, _b) in enumerate(BLOCKS):
    for _t in range(_a, _b):
        BLOCK_OF[_t] = _j
BLK_W = [(b - a) * BL for a, b in BLOCKS]          # 128..512 cols
BLK_G = [2048 // w for w in BLK_W]                  # k-tiles per DMA
BLK_ND = [NKTP // g for g in BLK_G]                 # DMAs per block

_prog_cache = {}


def _build_program():
    """One SPMD Bass program; every core runs it on its own 64-row batch shard."""
    nc = bacc.Bacc()

    d_xs = [nc.declare_dram_parameter(f"xk{j}", [BLK_ND[j], 128, 2048],
                                      BF16, isOutput=False)
            for j in range(NBLKV)]
    d_wn = nc.declare_dram_parameter("wn", [128, NKTP, H], BF16, isOutput=False)
    d_cp = nc.declare_dram_parameter("cpack", [128, CPW], BF16, isOutput=False)
    d_cb = nc.declare_dram_parameter("cbias", [128, CBW], F32, isOutput=False)
    d_wih = nc.declare_dram_parameter("wihT", [H, 7, 4 * H], BF16, isOutput=False)
    d_whh = nc.declare_dram_parameter("whhT", [H, 7, 4 * H], BF16, isOutput=False)
    d_w1 = nc.declare_dram_parameter("w1T", [H, 7, HU], BF16, isOutput=False)
    d_out = nc.declare_dram_parameter("out", [1, 7 * BL], F32, isOutput=True)

    ADD = mybir.AluOpType.add
    MUL = mybir.AluOpType.mult

    with tile.TileContext(nc) as tc:
        with (
            tc.tile_pool(name="consts", bufs=1) as consts,
            tc.tile_pool(name="xpool", bufs=5) as xpool,
            tc.tile_pool(name="gsb", bufs=12) as gsb,
            tc.tile_pool(name="psum_g", bufs=6, space="PSUM") as psum_g,
            tc.tile_pool(name="psum_gemm", bufs=2, space="PSUM") as psum_gemm,
        ):
            cp = consts.tile([128, CPW], BF16, tag="cp")
            nc.sync.dma_start(cp[:], d_cp[:])
            cb = consts.tile([128, CBW], F32, tag="cb")
            nc.sync.dma_start(cb[:], d_cb[:])
            wih_sb = consts.tile([H, 7, 4 * H], BF16, tag="wih")
            nc.sync.dma_start(wih_sb[:], d_wih[:])
            whh_sb = consts.tile([H, 7, 4 * H], BF16, tag="whh")
            nc.sync.dma_start(whh_sb[:], d_whh[:])
            wn_sb = consts.tile([128, NKTP, H], BF16, tag="wn")
            nc.sync.dma_start(wn_sb[:], d_wn[:])
            # w1 (head weights) queued later: not needed until the first head

            # ---- small projections ----
            tauh_sb = consts.tile([H, LCOLS], BF16, tag="tauh")
            for j in range(LCOLS // 512):
                ps = psum_gemm.tile([128, 512], F32, tag="gemm")
                nc.tensor.matmul(ps[:], cp[:, C_WTAU:C_WTAU + 128],
                                 cp[:, C_TAU + j * 512:C_TAU + (j + 1) * 512],
                                 start=True, stop=True)
                nc.scalar.activation(tauh_sb[:, j * 512:(j + 1) * 512], ps[:],
                                     TANH, bias=cb[:, Z_BTAU:Z_BTAU + 1])

            ps1_t = psum_gemm.tile([128, 512], F32, tag="gemm", name="ps1")
            ps1 = ps1_t[0:1, 0:BL]
            nc.tensor.matmul(ps1[:], cp[:, C_WX1:C_WX1 + 1],
                             cp[:, C_XIN:C_XIN + BL], start=True, stop=True)
            s1_sb = gsb.tile([128, BL], BF16, tag="svec")
            nc.vector.memset(s1_sb[:], 0.0)
            nc.vector.tensor_copy(s1_sb[0:1, :], ps1[:])
            ps2_t = psum_gemm.tile([128, 512], F32, tag="gemm", name="ps2")
            ps2 = ps2_t[:, 0:BL]
            nc.tensor.matmul(ps2[:], cp[:, C_WX2:C_WX2 + 128], s1_sb[:],
                             start=True, stop=True)
            xh_sb = consts.tile([H, BL], BF16, tag="xh")
            nc.scalar.activation(xh_sb[:], ps2[:], TANH,
                                 bias=cb[:, Z_BX2:Z_BX2 + 1])

            ps3_t = psum_gemm.tile([128, 512], F32, tag="gemm", name="ps3")
            ps3 = ps3_t[:, 0:BL]
            nc.tensor.matmul(ps3[:], cp[:, C_WRES:C_WRES + 128],
                             cp[:, C_T0:C_T0 + BL], start=True, stop=True)
            t0h_sb = consts.tile([H, BL], BF16, tag="t0h")
            nc.scalar.activation(t0h_sb[:], ps3[:], TANH,
                                 bias=cb[:, Z_BRES:Z_BRES + 1])

            ps4_t = psum_gemm.tile([128, 512], F32, tag="gemm", name="ps4")
            ps4 = ps4_t[0:1, 0:BL]
            nc.tensor.matmul(ps4[:], cp[:, C_WE1:C_WE1 + 1],
                             cp[:, C_END:C_END + BL], start=True, stop=True)
            s2_sb = gsb.tile([128, BL], BF16, tag="svec")
            nc.vector.memset(s2_sb[:], 0.0)
            nc.vector.tensor_copy(s2_sb[0:1, :], ps4[:])
            ps5_t = psum_gemm.tile([128, 512], F32, tag="gemm", name="ps5")
            ps5 = ps5_t[:, 0:BL]
            nc.tensor.matmul(ps5[:], cp[:, C_WE2:C_WE2 + 128], s2_sb[:],
                             start=True, stop=True)
            endh_sb = consts.tile([H, BL], BF16, tag="endh")
            nc.scalar.activation(endh_sb[:], ps5[:], IDENT,
                                 bias=cb[:, Z_BE2:Z_BE2 + 1])

            coordh_sb = consts.tile([H, COLS], BF16, tag="coordh")
            for j in range(COLS // 512):
                ps = psum_gemm.tile([128, 512], F32, tag="gemm")
                nc.tensor.matmul(ps[:], cp[:, C_WC:C_WC + 128],
                                 cp[:, C_COORDS + j * 512:C_COORDS + (j + 1) * 512],
                                 start=True, stop=True)
                nc.vector.tensor_copy(coordh_sb[:, j * 512:(j + 1) * 512], ps[:])

            # ---- materialized sequence buffers (bf16) ----
            # +4 slack positions so strided scatter views stay in-bounds
            seq_sb = {k: consts.tile([H, (len(SEQS[k]) + 4) * BL], BF16,
                                     tag=f"seq{k}", name=f"seq{k}")
                      for k in SEQ_MAT}

            def seq_dst(k, pos0, ngrp, stride):
                s = seq_sb[k]
                v = s[:, pos0 * BL:(pos0 + ngrp * stride) * BL]
                return v.rearrange("p (l g b) -> p l g b", l=ngrp,
                                   g=stride)[:, :, 0, :]

            def scopy(dst, src, strided=True):
                # strided 3D copies are ~2x slower on GpSimd; keep them on DVE
                eng = nc.vector if strided else nc.gpsimd
                eng.tensor_copy(dst, src)

            # tau / pre / suf scatter (sources ready early)
            scopy(seq_dst(0, 2, L, 5), tauh_sb[:].rearrange(
                "p (l b) -> p l b", l=L))
            scopy(seq_sb[1][:, 2 * BL:(2 + L) * BL], tauh_sb[:], strided=False)
            scopy(seq_dst(4, 2, L, 3), tauh_sb[:].rearrange(
                "p (l b) -> p l b", l=L))
            scopy(seq_dst(6, 2, L, 3), tauh_sb[:].rearrange(
                "p (l b) -> p l b", l=L))
            for k in (0, 1, 4, 6):
                tk = len(SEQS[k])
                scopy(seq_sb[k][:, 0:BL], xh_sb[:], strided=False)
                scopy(seq_sb[k][:, BL:2 * BL], t0h_sb[:], strided=False)
                scopy(seq_sb[k][:, (tk - 1) * BL:tk * BL], endh_sb[:],
                      strided=False)
            # coord scatter per 512-col block (4 l-groups each)
            for j in range(NBLK):
                cv = coordh_sb[:, j * 512:(j + 1) * 512].rearrange(
                    "p (l q b) -> p l q b", l=4, q=2)
                scopy(seq_dst(0, 4 + 20 * j, 4, 5), cv[:, :, 0, :])
                scopy(seq_dst(0, 6 + 20 * j, 4, 5), cv[:, :, 1, :])
                scopy(seq_dst(6, 3 + 12 * j, 4, 3), cv[:, :, 0, :])
                scopy(seq_dst(6, 4 + 12 * j, 4, 3), cv[:, :, 1, :])
                scopy(seq_dst(5, 16 * j + 1, 4, 4), cv[:, :, 0, :])
                scopy(seq_dst(5, 16 * j + 3, 4, 4), cv[:, :, 1, :])

            # ---- node GEMM: column-block-major, 4 k-tiles per DMA ----
            nodeh_blk = [consts.tile([H, (b - a) * BL], BF16, tag=f"nodeh{j}",
                                     name=f"nodeh{j}")
                         for j, (a, b) in enumerate(BLOCKS)]
            gemm_ps = {}

            def gemm_item(j, q):
                ta, tb = BLOCKS[j]
                w, G, nd = BLK_W[j], BLK_G[j], BLK_ND[j]
                if q == 0:
                    gemm_ps[j] = psum_gemm.tile([128, 512], F32, tag="gemm",
                                                name=f"gemm{j}")
                xt = xpool.tile([128, 2048], BF16, tag="xt")
                nc.sync.dma_start(xt[:], d_xs[j][q])
                for u in range(G):
                    nc.tensor.matmul(gemm_ps[j][:, 0:w], wn_sb[:, q * G + u],
                                     xt[:, u * w:(u + 1) * w],
                                     start=(q == 0 and u == 0),
                                     stop=(q == nd - 1 and u == G - 1))
                if q == nd - 1:
                    blk = nodeh_blk[j]
                    nc.vector.tensor_copy(blk[:], gemm_ps[j][:, 0:w])
                    ng, la = (tb - ta) // 2, ta // 2
                    nv = blk[:].rearrange("p (l q b) -> p l q b", l=ng, q=2)
                    scopy(seq_dst(0, 3 + 5 * la, ng, 5), nv[:, :, 0, :])
                    scopy(seq_dst(0, 5 + 5 * la, ng, 5), nv[:, :, 1, :])
                    scopy(seq_dst(4, 3 + 3 * la, ng, 3), nv[:, :, 0, :])
                    scopy(seq_dst(4, 4 + 3 * la, ng, 3), nv[:, :, 1, :])
                    scopy(seq_dst(5, 4 * la, ng, 4), nv[:, :, 0, :])
                    scopy(seq_dst(5, 4 * la + 2, ng, 4), nv[:, :, 1, :])

            gemm_work = [(j, q) for j in range(NBLKV) for q in range(BLK_ND[j])]
            gemm_pos = 0

            def pump_gemm_until_block(jneed):
                nonlocal gemm_pos
                while gemm_pos < len(gemm_work) and gemm_work[gemm_pos][0] <= jneed:
                    gemm_item(*gemm_work[gemm_pos])
                    gemm_pos += 1

            def pump_gemm(n):
                nonlocal gemm_pos
                for _ in range(n):
                    if gemm_pos >= len(gemm_work):
                        return
                    gemm_item(*gemm_work[gemm_pos])
                    gemm_pos += 1

            w1_sb = consts.tile([H, 7, HU], BF16, tag="w1")
            nc.sync.dma_start(w1_sb[:], d_w1[:])

            # ---- LSTM machinery ----
            state = {}
            for k in range(7):
                state[k] = dict(
                    h=consts.tile([H, BL], BF16, tag=f"h{k}", name=f"h{k}"),
                    c=consts.tile([H, BL], F32, tag=f"c{k}", name=f"c{k}"),
                )

            def ih_rhs(k, t, w):
                """moving operand for the ih matmul covering steps t..t+w/64/?"""
                if k == 2:
                    j = BLOCK_OF[t]
                    off = (t - BLOCKS[j][0]) * BL
                    return nodeh_blk[j][:, off:off + w]
                if k == 3:
                    return coordh_sb[:, t * BL:t * BL + w]
                return seq_sb[k][:, t * BL:t * BL + w]

            def req_blk(k, t):
                src, i = SEQS[k][t]
                return BLOCK_OF[i] if src == "node" else -1

            # LSTM 0 is the serial spine: keep its chain on DVE (lowest
            # latency); offload the gate products of the others to GpSimd.
            USE_GP = {0: False, 1: True, 2: True, 3: True, 4: True, 5: True, 6: True}

            pair_ps = {}

            def step(k, t):
                st = state[k]
                hT, cT = st["h"], st["c"]
                Tk = len(SEQS[k])
                half = t % 2
                if half == 0:
                    ps = psum_g.tile([128, 512], F32, tag="gates_ps")
                    pair_ps[k] = ps
                    both = t + 1 < Tk
                    w = 512 if both else 256
                    # ih first: data-gated, so the PSUM bank is only taken
                    # once this chain can actually run. start=True on g0
                    # clears has_written bank-wide; g1-3 first-write their
                    # cols; the bias matmul then accumulates everywhere.
                    ps2 = ps[:].rearrange("p (a b) -> p a b", a=2)
                    for g in range(4):
                        if both:
                            nc.tensor.matmul(ps2[:, :, g * BL:(g + 1) * BL],
                                             wih_sb[:, k, g * H:(g + 1) * H],
                                             ih_rhs(k, t, 2 * BL),
                                             start=(g == 0), stop=False)
                        else:
                            nc.tensor.matmul(ps[:, g * BL:(g + 1) * BL],
                                             wih_sb[:, k, g * H:(g + 1) * H],
                                             ih_rhs(k, t, BL),
                                             start=(g == 0), stop=False)
                    nc.tensor.matmul(ps[:, 0:w],
                                     cp[0:4, C_BK + k * 128:C_BK + (k + 1) * 128],
                                     cp[0:4, C_IND:C_IND + w],
                                     start=False, stop=(t == Tk - 1 and t == 0))
                else:
                    ps = pair_ps[k]
                base = half * 256
                last = (t == Tk - 1) or (half == 1)
                if t > 0:
                    for g in range(4):
                        nc.tensor.matmul(
                            ps[:, base + g * BL:base + (g + 1) * BL],
                            whh_sb[:, k, g * H:(g + 1) * H],
                            hT[:], start=False, stop=(last and g == 3))
                # one sigmoid over all four gates: g-gate weights/bias are
                # pre-scaled by 2 on the host, tanh(g) = 2*sig(2g) - 1, and
                # the *2-1 correction folds into the fused cell-update ops.
                gates = gsb.tile([128, 256], F32, tag="gates_sb")
                nc.scalar.activation(gates[:], ps[:, base:base + 256], SIG)
                eng = nc.gpsimd if USE_GP[k] else nc.vector
                SUB = mybir.AluOpType.subtract
                # scalar_tensor_tensor only lowers on DVE (not Pool)
                if t == 0:
                    ig = gsb.tile([128, BL], F32, tag="ig")
                    nc.vector.scalar_tensor_tensor(
                        ig[:], gates[:, 192:256], 0.5, gates[:, 0:BL], SUB, MUL)
                    nc.vector.tensor_scalar_mul(cT[:], ig[:], 2.0)
                else:
                    ig = gsb.tile([128, BL], F32, tag="ig")
                    nc.vector.scalar_tensor_tensor(
                        ig[:], gates[:, 192:256], 0.5, gates[:, 0:BL], SUB, MUL)
                    fc = gsb.tile([128, BL], F32, tag="fc")
                    eng.tensor_tensor(fc[:], gates[:, BL:2 * BL], cT[:], MUL)
                    nc.vector.scalar_tensor_tensor(cT[:], ig[:], 2.0, fc[:],
                                                   MUL, ADD)
                tcc = gsb.tile([128, BL], F32, tag="tanhc")
                nc.scalar.activation(tcc[:], cT[:], TANH)
                eng.tensor_tensor(hT[:], gates[:, 2 * BL:3 * BL], tcc[:], MUL)

            out_sb = consts.tile([1, 7 * BL], F32, tag="outsb")

            def head(k):
                hT = state[k]["h"]
                hp_t = psum_g.tile([128, 512], F32, tag="gates_ps", name="hp")
                hp = hp_t[:, 0:BL]
                nc.tensor.matmul(hp[:], w1_sb[:, k], hT[:], start=True, stop=True)
                z1 = gsb.tile([128, BL], BF16, tag="z1")
                nc.scalar.activation(z1[:], hp[:], TANH,
                                     bias=cb[:, Z_B1 + k:Z_B1 + k + 1])
                op_t = psum_g.tile([128, 512], F32, tag="gates_ps", name="op")
                op = op_t[0:1, 0:BL]
                nc.tensor.matmul(op[:], cp[:, C_W2 + k:C_W2 + k + 1], z1[:],
                                 start=True, stop=True)
                nc.scalar.activation(out_sb[:, k * BL:(k + 1) * BL], op[:],
                                     IDENT, bias=cb[0:1, Z_B2 + k:Z_B2 + k + 1])

            # ---- unified slot loop; GEMM items interleaved for overlap ----
            # staggered starts keep <=5 chains (PSUM gate banks) concurrent;
            # k5 (pure node) starts once the first node blocks exist
            order_steady = [0, 5, 4, 6, 2, 3, 1]
            # during the ramp, put node-free chains first so a stalled
            # node-dependent op can't head-of-line-block the engine FIFOs
            order_ramp = [6, 3, 1, 0, 4, 5, 2]
            # starts chosen so ~4 chains are active in every slot window and
            # late chains consume the spine's otherwise-idle tail
            START = {0: 0, 6: 0, 3: 0, 1: 0, 5: 19, 4: 30, 2: 49}
            nslots = max(START[k] + len(SEQS[k]) for k in range(7))
            pump_gemm_until_block(2)   # prefetch the first three blocks
            for s in range(nslots):
                pump_gemm(3)
                order = order_ramp if s < 16 else order_steady
                for k in order:
                    t = s - START[k]
                    if 0 <= t < len(SEQS[k]):
                        jneed = req_blk(k, t)
                        # even steps also emit the ih matmul for step t+1
                        if t % 2 == 0 and t + 1 < len(SEQS[k]):
                            jneed = max(jneed, req_blk(k, t + 1))
                        if jneed >= 0:
                            pump_gemm_until_block(jneed)
                        step(k, t)
                    elif t == len(SEQS[k]):
                        head(k)
            pump_gemm(len(gemm_work))
            for k in order_steady:
                if nslots == START[k] + len(SEQS[k]):
                    head(k)

            nc.sync.dma_start(d_out[:], out_sb[:])

    nc.finalize()
    return nc


def _get_program():
    if "nc" not in _prog_cache:
        _prog_cache["nc"] = _build_program()
    return _prog_cache["nc"]


def _pack_constants(inp):
    cpk = np.zeros((128, CPW), NPBF)
    cbk = np.zeros((128, CBW), np.float32)

    def put(dst, c, arr):
        dst[:arr.shape[0], c:c + arr.shape[1]] = arr

    put(cpk, C_WC, inp["Wcoord"].T)
    put(cpk, C_WTAU, inp["Wtau"].T)
    put(cpk, C_WX2, inp["Wx2"].T)
    put(cpk, C_WRES, inp["Wres"].T)
    put(cpk, C_WE2, inp["Wend2"].T)
    put(cpk, C_WX1, inp["Wx1"].T)
    put(cpk, C_WE1, inp["Wend1"].T)
    put(cpk, C_W2, inp["head_W2"].reshape(7, HU).T)
    # gate indicator [4, 512] (256-periodic): row p marks cols of gate-region p
    ind = np.zeros((4, 512), np.float32)
    for p in range(4):
        for rep in range(2):
            ind[p, rep * 256 + p * BL:rep * 256 + (p + 1) * BL] = 1.0
    put(cpk, C_IND, ind)
    # per-LSTM bias rows [4, 128] in packed gate order (i,f,o,g); g scaled by 2
    bsum = (inp["lstm_bih"] + inp["lstm_bhh"]).reshape(7, 4, H)[:, GPERM].copy()
    bsum[:, 3] *= 2.0
    for k in range(7):
        put(cpk, C_BK + k * 128, bsum[k])
    put(cbk, Z_BTAU, inp["btau"][:, None])
    put(cbk, Z_BX2, inp["bx2"][:, None])
    put(cbk, Z_BRES, inp["bres"][:, None])
    put(cbk, Z_BE2, inp["bend2"][:, None])
    put(cbk, Z_B1, inp["head_b1"].T)
    put(cbk, Z_B2, inp["head_b2"].reshape(1, 7))
    return cpk, cbk


def _make_in_maps(inp):
    node = inp["node_inputs"]
    coords = inp["coords"]
    tau = inp["tau_inputs"]
    x = inp["x"]
    t0 = inp["t0_res"]
    end = inp["end"]

    wn = np.zeros((NPAD, H), NPBF)
    wn[:N] = inp["Wnode"].T
    wn_dev = np.ascontiguousarray(wn.reshape(NKTP, 128, H).transpose(1, 0, 2))

    # reorder torch gate blocks (i,f,g,o) -> packed (i,f,o,g); the g gate is
    # scaled by 2 so tanh(g) can run through the shared sigmoid table
    wih_r = inp["lstm_Wih"].reshape(7, 4, H, H)[:, GPERM].copy()
    whh_r = inp["lstm_Whh"].reshape(7, 4, H, H)[:, GPERM].copy()
    wih_r[:, 3] *= 2.0
    whh_r[:, 3] *= 2.0
    wih_r = wih_r.reshape(7, 4 * H, H)
    whh_r = whh_r.reshape(7, 4 * H, H)
    wih = np.ascontiguousarray(wih_r.transpose(2, 0, 1).astype(NPBF))
    whh = np.ascontiguousarray(whh_r.transpose(2, 0, 1).astype(NPBF))
    w1 = np.ascontiguousarray(inp["head_W1"].transpose(2, 0, 1).astype(NPBF))

    cpk_base, cbk = _pack_constants(inp)

    in_maps = []
    for c in range(NCORES):
        sl = slice(c * BL, (c + 1) * BL)
        xk = np.zeros((NPAD, COLS), NPBF)
        xk[:N] = node[sl].transpose(2, 1, 0).reshape(N, COLS)
        xk_t = xk.reshape(NKTP, 128, COLS)
        xk_blocks = {}
        for j, (a, b) in enumerate(BLOCKS):
            w, G, nd = BLK_W[j], BLK_G[j], BLK_ND[j]
            arr = xk_t[:, :, a * BL:b * BL].reshape(nd, G, 128, w)
            xk_blocks[f"xk{j}"] = np.ascontiguousarray(
                arr.transpose(0, 2, 1, 3)).reshape(nd, 128, 2048)
        cpk = cpk_base.copy()
        cpk[:2, C_XIN:C_XIN + BL] = x[sl].T
        cpk[:1, C_T0:C_T0 + BL] = t0[sl].T
        cpk[:2, C_END:C_END + BL] = end[sl].T
        cpk[:1, C_TAU:C_TAU + LCOLS] = tau[sl].transpose(2, 1, 0).reshape(1, LCOLS)
        cpk[:2, C_COORDS:C_COORDS + COLS] = coords[sl].transpose(2, 1, 0).reshape(2, COLS)
        in_maps.append(dict(
            wn=wn_dev, cpack=cpk, cbias=cbk, wihT=wih, whhT=whh, w1T=w1,
            **xk_blocks,
        ))
    return in_maps


def kernel(**inputs):
    inp = {k: np.asarray(v, dtype=np.float32) for k, v in inputs.items()}
    in_maps = _make_in_maps(inp)
    nc = _get_program()
    res = run_bass_kernel_spmd(nc, in_maps, core_ids=list(range(NCORES)))
    if res.exec_time_ns is not None:
        print(f"HW exec time: {res.exec_time_ns} ns")

    outs = [r["out"].reshape(7, BL) for r in res.results]
    full = np.concatenate(outs, axis=1)      # [7, B]
    return tuple(full[k][:, None].astype(np.float32) for k in range(7))
